# revision 11
# baseline (speedup 1.0000x reference)
"""Trainium2 Bass kernel for PCT-style point-cloud transformer (nn_GT_87625922773239).

Sharding: data-parallel over batch (8 batches -> 8 NeuronCores).
Phase 1 NEFF: kNN (exact top-20 via DVE max8/max_index/match_replace), neighbor
gathers (gpsimd indirect_copy column gathers in channel-major layout), edge MLPs,
K-neighbor attention with log-softmax, fc2. Outputs attn + res2 + x1.
Host: BatchNorm1 stats (cross-batch reduction) + relu + residual (elementwise).
Phase 2 NEFF: n x n offset self-attention (energy, softmax, column-normalize via
matmul-folded reciprocal), sa_t projection. Host: BatchNorm2 + final residual.
"""
import numpy as np

import concourse.bass as bass
import concourse.bacc as bacc
import concourse.mybir as mybir
from concourse.tile import TileContext
from concourse.bass_utils import run_bass_kernel_spmd
from concourse import library_config

F32 = mybir.dt.float32
F32R = mybir.dt.float32r
U16 = mybir.dt.uint16
AF = mybir.ActivationFunctionType
ALU = mybir.AluOpType
AX = mybir.AxisListType

B, DP, N = 8, 3, 2048
DM, K = 128, 20
NT = N // 128          # 16 point tiles
E = 128 * K            # 2560 edges per tile
SQ = float(np.sqrt(DM))

_cache = {}


def _r(ap):
    return ap  # fp32 matmuls (f32r needs producer-side rounding; revisit)


def _build_phase1():
    nc = bacc.Bacc("TRN2", target_bir_lowering=False, debug=False, num_devices=8)
    # ---- I/O ----
    feat = nc.dram_tensor("feat", [DP, N], F32, kind="ExternalInput")
    w_in = {}
    for name, shape in [
        ("fc1_wT", [DP, DM]), ("AT", [DP, DM]), ("BmT", [DP, DM]),
        ("wqT", [DM, DM]), ("wkT", [DM, DM]), ("wvT", [DM, DM]),
        ("fb2_wT", [DM, DM]), ("fg1_wT", [DM, DM]), ("fg2_wT", [DM, DM]),
        ("fc2_wT", [DM, DM]), ("ident", [DM, DM]),
        ("fc1_b", [DM, 1]), ("fb1_b", [DM, 1]), ("fb2_b", [DM, 1]),
        ("fg1_b", [DM, 1]), ("fg2_b_s", [DM, 1]), ("fc2_b", [DM, 1]),
    ]:
        w_in[name] = nc.dram_tensor(name, shape, F32, kind="ExternalInput")
    attn_out = nc.dram_tensor("attn_out", [N * K, DM], F32, kind="ExternalOutput")
    res2_out = nc.dram_tensor("res2_out", [DM, N], F32, kind="ExternalOutput")
    x1t_out = nc.dram_tensor("x1t_out", [DM, N], F32, kind="ExternalOutput")
    idx_hbm = nc.dram_tensor("idx_hbm", [NT, E], U16, kind="Internal")

    with TileContext(nc) as tc:
        with (
            tc.tile_pool(name="const", bufs=1) as cp,
            tc.tile_pool(name="tab", bufs=1) as tp,
            tc.tile_pool(name="big", bufs=1, space="PSUM") as bigp,
            tc.tile_pool(name="mm", bufs=2, space="PSUM") as mmp,
            tc.tile_pool(name="tr", bufs=2, space="PSUM") as trp,
            tc.tile_pool(name="edge", bufs=1) as ep,
            tc.tile_pool(name="small", bufs=2) as sp,
        ):
            W = {k: cp.tile(list(v.shape), F32, tag=k, name=k) for k, v in w_in.items()}
            for k, v in w_in.items():
                nc.sync.dma_start(W[k][:], v[:])

            x_sb = tp.tile([DP, N], F32, tag="x")
            nc.sync.dma_start(x_sb[:], feat[:])
            nc.gpsimd.load_library(library_config.ap_gather)

            # ---- selection scores s[i,j] = 2 x_i.x_j - xx_j  (= pd + const(i)) ----
            sq = tp.tile([DP, N], F32, tag="sq")
            nc.vector.tensor_tensor(sq[:], x_sb[:], x_sb[:], op=ALU.mult)
            ones3 = cp.tile([DP, 1], F32, tag="ones3")
            nc.vector.memset(ones3[:], 1.0)
            xx_ps = bigp.tile([1, N], F32, tag="big")
            for c in range(4):
                s = slice(c * 512, (c + 1) * 512)
                nc.tensor.matmul(xx_ps[:, s], ones3[:], sq[:, s], start=True, stop=True)
            la = tp.tile([4, N], F32, tag="la")
            xa = tp.tile([4, N], F32, tag="xa")
            nc.vector.memset(la[:], 1.0)
            nc.vector.tensor_copy(la[0:DP, :], x_sb[:])
            nc.vector.tensor_scalar_mul(xa[0:DP, :], x_sb[:], 2.0)
            negxx = tp.tile([1, N], F32, tag="negxx")
            nc.scalar.activation(negxx[:], xx_ps[:], AF.Copy, scale=-1.0)
            nc.sync.dma_start(xa[DP:4, :], negxx[:])

            # ---- tables (channel-major [DM, N]) ----
            x1T = tp.tile([DM, N], F32, tag="x1T")
            AxT = tp.tile([DM, N], F32, tag="AxT")
            BxT = tp.tile([DM, N], F32, tag="BxT")
            qT = tp.tile([DM, N], F32, tag="qT")
            kT = tp.tile([DM, N], F32, tag="kT")
            vT = tp.tile([DM, N], F32, tag="vT")
            for c in range(4):
                s = slice(c * 512, (c + 1) * 512)
                ps = mmp.tile([128, 512], F32, tag="mm")
                nc.tensor.matmul(ps[:], W["fc1_wT"][:], x_sb[:, s], start=True, stop=True)
                nc.scalar.activation(x1T[:, s], ps[:], AF.Identity, bias=W["fc1_b"][:])
                ps2 = mmp.tile([128, 512], F32, tag="mm")
                nc.tensor.matmul(ps2[:], W["AT"][:], x_sb[:, s], start=True, stop=True)
                nc.scalar.activation(AxT[:, s], ps2[:], AF.Copy)
                ps3 = mmp.tile([128, 512], F32, tag="mm")
                nc.tensor.matmul(ps3[:], W["BmT"][:], x_sb[:, s], start=True, stop=True)
                nc.scalar.activation(BxT[:, s], ps3[:], AF.Copy)
            for c in range(4):
                s = slice(c * 512, (c + 1) * 512)
                for wname, dst in [("wqT", qT), ("wkT", kT), ("wvT", vT)]:
                    ps = mmp.tile([128, 512], F32, tag="mm")
                    nc.tensor.matmul(ps[:], _r(W[wname][:]), _r(x1T[:, s]), start=True, stop=True)
                    nc.scalar.activation(dst[:, s], ps[:], AF.Copy)
            nc.sync.dma_start(x1t_out[:], x1T[:])

            res_sb = tp.tile([DM, N], F32, tag="res")

            for t in range(NT):
                tsl = slice(t * 128, (t + 1) * 128)
                # ---- pairwise scores + exact top-20 ----
                pd_ps = bigp.tile([128, N], F32, tag="big")
                for c in range(4):
                    s = slice(c * 512, (c + 1) * 512)
                    nc.tensor.matmul(pd_ps[:, s], la[:, tsl], xa[:, s], start=True, stop=True)
                pd = ep.tile([128, N], F32, tag="pd")
                nc.scalar.activation(pd[:], pd_ps[:], AF.Copy)
                gidx = sp.tile([128, 24], U16, tag="gidx")
                m8 = sp.tile([128, 8], F32, tag="m8")
                for r in range(3):
                    nc.vector.max(m8[:], pd[:])
                    nc.vector.max_index(gidx[:, r * 8:(r + 1) * 8], m8[:], pd[:])
                    if r < 2:
                        nc.vector.match_replace(pd[:], m8[:], pd[:], -1e30)
                # idx -> HBM (edge order), read back wrapped+replicated
                nc.sync.dma_start(idx_hbm[t, :], gidx[:, :K])
                irep = sp.tile([128, E // 16], U16, tag="irep")
                src = idx_hbm[t, :].rearrange("(s q) -> q s", q=16)
                for g in range(8):
                    nc.sync.dma_start(irep[16 * g:16 * (g + 1), :], src)

                # ---- gathers (channel-major column gather) ----
                Axg = ep.tile([128, E], F32, tag="Axg")
                kg = ep.tile([128, E], F32, tag="kg")
                vg = ep.tile([128, E], F32, tag="vg")
                irep_i = irep[:].bitcast(mybir.dt.int16)
                nc.gpsimd.ap_gather(Axg[:], AxT[:], irep_i, 128, N, 1, E)
                nc.gpsimd.ap_gather(kg[:], kT[:], irep_i, 128, N, 1, E)
                nc.gpsimd.ap_gather(vg[:], vT[:], irep_i, 128, N, 1, E)

                # ---- edge MLP: kf = fb2 @ relu(Axg + Bx + b1) + b2 ----
                bx_b = BxT[:, tsl].broadcast_to([128, 128, K])
                nc.vector.tensor_tensor(Axg[:].rearrange("p (n k) -> p n k", k=K),
                                        Axg[:].rearrange("p (n k) -> p n k", k=K),
                                        bx_b, op=ALU.add)
                relu1 = ep.tile([128, E], F32, tag="relu1")
                nc.scalar.activation(relu1[:], Axg[:], AF.Relu, bias=W["fb1_b"][:])
                kf = ep.tile([128, E], F32, tag="kf")
                for c in range(5):
                    s = slice(c * 512, (c + 1) * 512)
                    ps = mmp.tile([128, 512], F32, tag="mm")
                    nc.tensor.matmul(ps[:], _r(W["fb2_wT"][:]), _r(relu1[:, s]), start=True, stop=True)
                    nc.scalar.activation(kf[:, s], ps[:], AF.Identity, bias=W["fb2_b"][:])

                # ---- a = q - kg + kf ; gamma MLP ----
                q_b = qT[:, tsl].broadcast_to([128, 128, K])
                kg3 = kg[:].rearrange("p (n k) -> p n k", k=K)
                nc.vector.tensor_tensor(kg3, q_b, kg3, op=ALU.subtract)
                nc.vector.tensor_tensor(kg[:], kg[:], kf[:], op=ALU.add)
                for c in range(5):
                    s = slice(c * 512, (c + 1) * 512)
                    ps = mmp.tile([128, 512], F32, tag="mm")
                    nc.tensor.matmul(ps[:], _r(W["fg1_wT"][:]), _r(kg[:, s]), start=True, stop=True)
                    nc.scalar.activation(relu1[:, s], ps[:], AF.Relu, bias=W["fg1_b"][:])
                exp_sb = ep.tile([128, E], F32, tag="exp")
                a_s = ep.tile([128, E], F32, tag="a_s")
                for c in range(5):
                    s = slice(c * 512, (c + 1) * 512)
                    ps = mmp.tile([128, 512], F32, tag="mm")
                    nc.tensor.matmul(ps[:], _r(W["fg2_wT"][:]), _r(relu1[:, s]), start=True, stop=True)
                    nc.scalar.activation(exp_sb[:, s], ps[:], AF.Exp, bias=W["fg2_b_s"][:], scale=1.0 / SQ)
                    nc.scalar.activation(a_s[:, s], ps[:], AF.Identity, bias=W["fg2_b_s"][:], scale=1.0 / SQ)

                # ---- log-softmax over K + res ----
                sume = sp.tile([128, 128], F32, tag="sume")
                nc.vector.tensor_reduce(sume[:], exp_sb[:].rearrange("p (n k) -> p n k", k=K),
                                        axis=AX.X, op=ALU.add)
                lns = sp.tile([128, 128], F32, tag="lns")
                nc.scalar.activation(lns[:], sume[:], AF.Ln)
                a3 = a_s[:].rearrange("p (n k) -> p n k", k=K)
                nc.vector.tensor_tensor(a3, a3, lns[:].broadcast_to([128, 128, K]), op=ALU.subtract)
                # w = vg + kf ; prod = attn * w ; res_tile = sum_k prod
                nc.vector.tensor_tensor(vg[:], vg[:], kf[:], op=ALU.add)
                nc.vector.tensor_tensor(vg[:], vg[:], a_s[:], op=ALU.mult)
                nc.vector.tensor_reduce(res_sb[:, tsl], vg[:].rearrange("p (n k) -> p n k", k=K),
                                        axis=AX.X, op=ALU.add)

                # ---- attn output: transpose 128-blocks -> HBM ----
                for g in range(K):
                    trp_t = trp.tile([128, 128], F32, tag="tr")
                    nc.tensor.transpose(trp_t[:], a_s[:, g * 128:(g + 1) * 128], W["ident"][:])
                    atr = sp.tile([128, 128], F32, tag="atr", bufs=4)
                    nc.scalar.activation(atr[:], trp_t[:], AF.Copy)
                    nc.sync.dma_start(attn_out[t * E + g * 128:t * E + (g + 1) * 128, :], atr[:])

            # ---- fc2 ----
            for c in range(4):
                s = slice(c * 512, (c + 1) * 512)
                ps = mmp.tile([128, 512], F32, tag="mm")
                nc.tensor.matmul(ps[:], _r(W["fc2_wT"][:]), _r(res_sb[:, s]), start=True, stop=True)
                nc.scalar.activation(x1T[:, s], ps[:], AF.Identity, bias=W["fc2_b"][:])
            nc.sync.dma_start(res2_out[:], x1T[:])
    nc.compile()
    return nc


def _build_phase2():
    nc = bacc.Bacc("TRN2", target_bir_lowering=False, debug=False, num_devices=8)
    res_in = nc.dram_tensor("res_in", [DM, N], F32, kind="ExternalInput")
    w_in = {}
    for name, shape in [
        ("sa_qk_wT", [DM, 32]), ("sa_v_wT", [DM, DM]), ("sa_t_wT", [DM, DM]),
        ("sa_v_b_row", [1, DM]), ("sa_t_b", [DM, 1]), ("ones1", [1, DM]),
    ]:
        w_in[name] = nc.dram_tensor(name, shape, F32, kind="ExternalInput")
    xr2_out = nc.dram_tensor("xr2_out", [DM, N], F32, kind="ExternalOutput")

    with TileContext(nc) as tc:
        with (
            tc.tile_pool(name="const", bufs=1) as cp,
            tc.tile_pool(name="tab", bufs=1) as tp,
            tc.tile_pool(name="att", bufs=1) as ap_,
            tc.tile_pool(name="small", bufs=1) as sp,
        ):
            W = {k: cp.tile(list(v.shape), F32, tag=k, name=k) for k, v in w_in.items()}
            for k, v in w_in.items():
                nc.sync.dma_start(W[k][:], v[:])
            res = tp.tile([DM, N], F32, tag="res")
            nc.sync.dma_start(res[:], res_in[:])

            xqk = tp.tile([32, N], F32, tag="xqk")
            atts, recips, xvs = [], [], []
            with (
                tc.tile_pool(name="eps", bufs=1, space="PSUM") as epp,
                tc.tile_pool(name="mm", bufs=2, space="PSUM") as mmp,
            ):
                for c in range(4):
                    s = slice(c * 512, (c + 1) * 512)
                    ps = mmp.tile([32, 512], F32, tag="mm32")
                    nc.tensor.matmul(ps[:], _r(W["sa_qk_wT"][:]), _r(res[:, s]), start=True, stop=True)
                    nc.scalar.activation(xqk[:, s], ps[:], AF.Copy)

                for t in range(NT):
                    tsl = slice(t * 128, (t + 1) * 128)
                    e_ps = epp.tile([128, N], F32, tag="eps")
                    for c in range(4):
                        s = slice(c * 512, (c + 1) * 512)
                        nc.tensor.matmul(e_ps[:, s], _r(xqk[:, tsl]), _r(xqk[:, s]), start=True, stop=True)
                    att_t = ap_.tile([128, N], F32, tag=f"att{t}")
                    rs = sp.tile([128, 1], F32, tag=f"rs{t}")
                    nc.scalar.activation(att_t[:], e_ps[:], AF.Exp, accum_out=rs[:])
                    rc = sp.tile([128, 1], F32, tag=f"rc{t}")
                    nc.vector.reciprocal(rc[:], rs[:])
                    xv_ps = mmp.tile([128, 128], F32, tag="mmv")
                    nc.tensor.matmul(xv_ps[:], _r(res[:, tsl]), _r(W["sa_v_wT"][:]), start=True, stop=True)
                    xv_t = sp.tile([128, 128], F32, tag=f"xv{t}")
                    nc.vector.tensor_scalar_mul(xv_t[:], xv_ps[:], rc[:])
                    atts.append(att_t); recips.append(rc); xvs.append(xv_t)

            cs_sb = sp.tile([1, N], F32, tag="cs_sb")
            r2 = sp.tile([1, N], F32, tag="r2")
            with tc.tile_pool(name="csps", bufs=1, space="PSUM") as csp:
                cs_ps = csp.tile([1, N], F32, tag="cs")
                for t in range(NT):
                    for c in range(4):
                        s = slice(c * 512, (c + 1) * 512)
                        nc.tensor.matmul(cs_ps[:, s], _r(recips[t][:]), _r(atts[t][:, s]),
                                         start=(t == 0), stop=(t == NT - 1))
                nc.scalar.activation(cs_sb[:], cs_ps[:], AF.Copy)
                nc.vector.tensor_scalar_add(r2[:], cs_sb[:], 1e-9)
                nc.vector.reciprocal(r2[:], r2[:])

            r2b = tp.tile([128, N], F32, tag="r2b")
            xr = tp.tile([128, N], F32, tag="xr_sb")
            with tc.tile_pool(name="xrps", bufs=1, space="PSUM") as xrp:
                xr_ps = xrp.tile([128, N], F32, tag="xr")
                for t in range(NT):
                    for c in range(4):
                        s = slice(c * 512, (c + 1) * 512)
                        nc.tensor.matmul(xr_ps[:, s], _r(xvs[t][:]), _r(atts[t][:, s]),
                                         start=(t == 0), stop=False)
                for c in range(4):
                    s = slice(c * 512, (c + 1) * 512)
                    nc.tensor.matmul(xr_ps[:, s], _r(W["sa_v_b_row"][:]), _r(cs_sb[:, s]), start=False, stop=True)
                r2b_ps = xrp.tile([128, N], F32, tag="r2bp")
                for c in range(4):
                    s = slice(c * 512, (c + 1) * 512)
                    nc.tensor.matmul(r2b_ps[:, s], _r(W["ones1"][:]), _r(r2[:, s]), start=True, stop=True)
                nc.scalar.activation(r2b[:], r2b_ps[:], AF.Copy)
                nc.vector.scalar_tensor_tensor(xr[:], xr_ps[:], 1.0, r2b[:], op0=ALU.mult, op1=ALU.mult)
            nc.vector.tensor_tensor(xr[:], res[:], xr[:], op=ALU.subtract)
            with tc.tile_pool(name="mm2", bufs=2, space="PSUM") as mm2:
                for c in range(4):
                    s = slice(c * 512, (c + 1) * 512)
                    ps = mm2.tile([128, 512], F32, tag="mm2")
                    nc.tensor.matmul(ps[:], _r(W["sa_t_wT"][:]), _r(xr[:, s]), start=True, stop=True)
                    nc.scalar.activation(r2b[:, s], ps[:], AF.Identity, bias=W["sa_t_b"][:])
            nc.sync.dma_start(xr2_out[:], r2b[:])
    nc.compile()
    return nc


def _bn_host(x, g, b, eps=1e-5):
    # x: (B, C, N) fp32 -> train-mode BN over (batch, points)
    x64 = x.astype(np.float64)
    mu = x64.mean(axis=(0, 2), keepdims=True)
    var = ((x64 - mu) ** 2).mean(axis=(0, 2), keepdims=True)
    out = g[None, :, None] * (x64 - mu) / np.sqrt(var + eps) + b[None, :, None]
    return out.astype(np.float32)


def kernel(**inputs):
    inp = {k: np.ascontiguousarray(np.asarray(v, np.float32)) for k, v in inputs.items()}
    if "nc1" not in _cache:
        _cache["nc1"] = _build_phase1()
        _cache["nc2"] = _build_phase2()
    nc1, nc2 = _cache["nc1"], _cache["nc2"]

    fb1 = inp["fb1_w"]  # (DM, 2*DP)
    w1 = {
        "fc1_wT": inp["fc1_w"].T, "AT": fb1[:, :DP].T, "BmT": (fb1[:, DP:] - fb1[:, :DP]).T,
        "wqT": inp["wq"].T, "wkT": inp["wk"].T, "wvT": inp["wv"].T,
        "fb2_wT": inp["fb2_w"].T, "fg1_wT": inp["fg1_w"].T, "fg2_wT": inp["fg2_w"].T,
        "fc2_wT": inp["fc2_w"].T, "ident": np.eye(DM, dtype=np.float32),
        "fc1_b": inp["fc1_b"].reshape(DM, 1), "fb1_b": inp["fb1_b"].reshape(DM, 1),
        "fb2_b": inp["fb2_b"].reshape(DM, 1), "fg1_b": inp["fg1_b"].reshape(DM, 1),
        "fg2_b_s": (inp["fg2_b"] / SQ).reshape(DM, 1), "fc2_b": inp["fc2_b"].reshape(DM, 1),
    }
    w1 = {k: np.ascontiguousarray(v, dtype=np.float32) for k, v in w1.items()}
    maps1 = [dict(w1, feat=inp["features"][b]) for b in range(B)]
    kw = {}
    import time as _t; _s = _t.time(); rr1 = run_bass_kernel_spmd(nc1, maps1, core_ids=list(range(B)), **kw); _cache['w1'] = _t.time() - _s
    _cache["t1"] = rr1.exec_time_ns
    r1 = rr1.results

    attn = np.stack([r["attn_out"] for r in r1]).reshape(B, N, K, DM)
    res2 = np.stack([r["res2_out"] for r in r1])   # (B, DM, N)
    x1t = np.stack([r["x1t_out"] for r in r1])
    res = np.maximum(_bn_host(res2, inp["bn_g"], inp["bn_b"]), 0.0) + x1t

    w2 = {
        "sa_qk_wT": inp["sa_qk_w"].T, "sa_v_wT": inp["sa_v_w"].T, "sa_t_wT": inp["sa_t_w"].T,
        "sa_v_b_row": inp["sa_v_b"].reshape(1, DM), "sa_t_b": inp["sa_t_b"].reshape(DM, 1),
        "ones1": np.ones((1, DM), np.float32),
    }
    w2 = {k: np.ascontiguousarray(v, dtype=np.float32) for k, v in w2.items()}
    maps2 = [dict(w2, res_in=res[b]) for b in range(B)]
    _s = _t.time(); rr2 = run_bass_kernel_spmd(nc2, maps2, core_ids=list(range(B)), **kw); _cache['w2'] = _t.time() - _s
    _cache["t2"] = rr2.exec_time_ns
    r2 = rr2.results
    xr2 = np.stack([r["xr2_out"] for r in r2])     # (B, DM, N)

    xr = np.maximum(_bn_host(xr2, inp["sa_bn_g"], inp["sa_bn_b"]), 0.0)
    out = np.transpose(res + xr, (0, 2, 1)).astype(np.float32)  # (B, N, DM)
    return out, attn
